# revision 10
# baseline (speedup 1.0000x reference)
"""AlphaKnot GNN-transformer kernel for 8 TRN2 NeuronCores (Bass/Tile).

Sharding: data-parallel over graphs — core c owns nodes [c*8192, (c+1)*8192)
(16 graphs x 512 nodes). Residual stream kept on-chip transposed
(features-on-partitions, bf16). Per layer: AllGather x rows -> indirect-DMA
neighbor gather -> xbar transpose -> K/logits (PE) -> global softmax-sum
AllReduce (20 scalars) -> V/Z -> LN -> FFN -> LN. Heads at the end.
"""
import contextlib
import math

import numpy as np
import ml_dtypes

import concourse.bass as bass
import concourse.bacc as bacc
import concourse.mybir as mybir
import concourse.tile as tile
from concourse.masks import make_identity
from concourse.bass_utils import run_bass_kernel_spmd

BF = mybir.dt.bfloat16
F32 = mybir.dt.float32
I32 = mybir.dt.int32
AF = mybir.ActivationFunctionType
OP = mybir.AluOpType

# problem dims (hardcoded; kernel.py must be self-contained)
L, H, D, DK, DV = 4, 4, 256, 64, 64
DFF, MOVES, VDIM, PDIM = 1024, 14, 512, 512
B, NPG = 128, 512
N = B * NPG            # 65536
NCORES = 8
M = N // NCORES        # 8192 nodes per core
P = 128
NCH = 16               # n-chunks of 512 per core
CB = 512               # chunk width
R = 5                  # relations (self + 4 neighbors)
D_PE = 128
MAX_LEN, PE_N = 2000, 10000.0
SCALE = 1.0 / math.sqrt(DK)
EPS = 1e-5

_cached = {}
DEBUG = False


# ---------------------------------------------------------------- host prep
def _make_pe():
    pos = np.arange(MAX_LEN, dtype=np.float32)[:, None]
    div = np.exp(np.arange(0, D_PE, 2, dtype=np.float32) * (-math.log(PE_N) / D_PE))
    ang = (pos * div).astype(np.float32)
    return np.stack([np.sin(ang), np.cos(ang)], axis=-1).reshape(MAX_LEN, D_PE).astype(np.float32)


def _host_x0(dowker, ptr):
    n2 = dowker.shape[0]
    n = n2 // 2
    pe = _make_pe()
    counts = ptr[1:] - ptr[:-1]
    shifts = np.repeat(ptr[:-1] * 2, counts * 2)
    local_seq = np.arange(n2) - shifts
    pe_vecs = pe[local_seq]
    flat_idx = dowker[:, 0] * 2 + dowker[:, 1]
    out_flat = np.zeros((n2, D_PE), np.float32)
    out_flat[flat_idx] = pe_vecs
    return out_flat.reshape(n, 2 * D_PE)


def _prep_weights(inp):
    """Transpose weights to [in, out] (lhsT) layouts, numpy f32."""
    w = {}
    for l in range(L):
        w[f"wqT{l}"] = np.ascontiguousarray(np.transpose(inp["wq"][l], (1, 0, 2)).reshape(D, H * DK))
        w[f"wkT{l}"] = np.ascontiguousarray(np.transpose(inp["wk"][l], (1, 2, 0, 3)).reshape(R, D, H * DK))
        w[f"wvT{l}"] = np.ascontiguousarray(np.transpose(inp["wv"][l], (1, 2, 0, 3)).reshape(R, D, H * DV))
        w[f"f1{l}"] = np.ascontiguousarray(inp["ffn_w1"][l])          # [256,1024]
        w[f"f1b{l}"] = np.ascontiguousarray(inp["ffn_b1"][l])         # [1024]
        w[f"f2{l}"] = np.ascontiguousarray(inp["ffn_w2"][l])          # [1024,256]
        w[f"f2b{l}"] = np.ascontiguousarray(inp["ffn_b2"][l])         # [256]
        for nm in ("ln1_g", "ln1_b", "ln2_g", "ln2_b"):
            w[f"{nm}{l}"] = np.ascontiguousarray(inp[nm][l])          # [256]
    w["pw1T"] = np.ascontiguousarray(inp["pw1"])   # [256,512]
    w["pb1"] = np.ascontiguousarray(inp["pb1"])
    w["pw2T"] = np.ascontiguousarray(inp["pw2"])   # [512,14]
    w["pb2"] = np.ascontiguousarray(inp["pb2"])
    w["vw1T"] = np.ascontiguousarray(inp["vw1"])   # [256,512]
    w["vb1"] = np.ascontiguousarray(inp["vb1"])
    w["vw2T"] = np.ascontiguousarray(inp["vw2"])   # [512,1]
    w["vb2"] = np.ascontiguousarray(inp["vb2"])
    return w


# ---------------------------------------------------------------- bass build
def build_bass():
    nc = bacc.Bacc(None, target_bir_lowering=False)

    prm = {}
    def par(name, shape, dt):
        prm[name] = nc.declare_dram_parameter(name, list(shape), dt, isOutput=False)
        return prm[name]

    par("xT0", (2, P, M), BF)
    par("xfull0", (N, D), BF)
    par("gidx", (P, 256), I32)
    par("c_ones1m", (1, P), BF)      # ones row (K=1 bcast lhsT)
    par("c_inv256", (P, 1), BF)      # 1/256 column (stats lhsT)
    par("c_ones2h", (P, 2), BF)      # h-block indicator (logits lhsT)
    par("c_ind2", (2, P), BF)        # h-block indicator rows (abc lhsT)
    for l in range(L):
        par(f"wqT{l}", (D, 256), F32)
        par(f"wkT{l}", (R, D, 256), F32)
        par(f"wvT{l}", (R, D, 256), F32)
        par(f"f1{l}", (D, DFF), F32)
        par(f"f1b{l}", (DFF,), F32)
        par(f"f2{l}", (DFF, D), F32)
        par(f"f2b{l}", (D,), F32)
        for nm in ("ln1_g", "ln1_b", "ln2_g", "ln2_b"):
            par(f"{nm}{l}", (D,), F32)
    par("pw1T", (D, PDIM), F32); par("pb1", (PDIM,), F32)
    par("pw2T", (PDIM, MOVES), F32); par("pb2", (MOVES,), F32)
    par("vw1T", (D, VDIM), F32); par("vb1", (VDIM,), F32)
    par("vw2T", (VDIM, 1), F32); par("vb2", (1,), F32)

    polT = nc.declare_dram_parameter("policyT", [MOVES, M], F32, isOutput=True)
    valT = nc.declare_dram_parameter("valT", [1, B // NCORES], F32, isOutput=True)
    dbg = {}
    if DEBUG:
        for nm, shape in [("xnb00", (P, CB)), ("qsb0", (P, CB)), ("prod0", (P, CB)),
                          ("ec0", (2, CB)), ("sp00", (2, NCH)), ("ab0", (2, P)),
                          ("absb0", (P, CB)), ("tmp0", (P, CB)), ("T1c0", (P, CB)),
                          ("xn1c0", (P, CB)), ("hc0", (P, CB)), ("T2c0", (P, CB)),
                          ("xTn0", (P, CB)), ("agin0", (P, D)), ("agout0", (P, D)),
                          ("agoutc4", (P, D)), ("xnb10", (P, CB))]:
            dbg[nm] = nc.declare_dram_parameter("dbg_" + nm, list(shape), F32, isOutput=True)

    with tile.TileContext(nc) as tc:
        _emit(nc, tc, prm, polT, valT, dbg)
    nc.compile()
    return nc


def _emit(nc, tc, prm, polT, valT, dbg=None):
    stack = contextlib.ExitStack()
    sb = stack.enter_context(tc.tile_pool(name="sb", bufs=2))
    acc = stack.enter_context(tc.tile_pool(name="acc", bufs=1))
    ps = stack.enter_context(tc.tile_pool(name="ps", bufs=2, space="PSUM"))
    dr = stack.enter_context(tc.tile_pool(name="dr", bufs=1, space="DRAM"))

    # ---- persistent constants
    idx_t = acc.tile([P, 256], I32)
    nc.sync.dma_start(idx_t[:], prm["gidx"][:])
    ones1m = acc.tile([1, P], BF)
    nc.sync.dma_start(ones1m[:], prm["c_ones1m"][:])
    inv256 = acc.tile([P, 1], BF)
    nc.sync.dma_start(inv256[:], prm["c_inv256"][:])
    ones2h = acc.tile([P, 2], BF)
    nc.sync.dma_start(ones2h[:], prm["c_ones2h"][:])
    ind2 = acc.tile([2, P], BF)
    nc.sync.dma_start(ind2[:], prm["c_ind2"][:])
    eps_t = acc.tile([1, 1], F32)
    nc.vector.memset(eps_t[:], EPS)
    ident = acc.tile([P, P], BF)
    make_identity(nc, ident[:])

    def dump(nm, ap):
        if dbg and nm in dbg:
            t = sb.tile(list(ap.shape), F32, tag="dbgt", bufs=2, name=f"dbg{nm}")
            nc.vector.tensor_copy(t[:], ap)
            nc.sync.dma_start(dbg[nm][:], t[:])

    # ---- residual stream xT: 2 chunk tiles [128, M] bf16
    xT = [sb.tile([P, M], BF, tag="xT", bufs=3, name=f"xT_in{c}") for c in range(2)]
    for c in range(2):
        nc.sync.dma_start(xT[c][:], prm["xT0"][c])

    def load_w(l):
        w = {}
        wq = [sb.tile([P, 256], BF, tag="wq", bufs=2, name=f"wq{l}_{dc}") for dc in range(2)]
        for dc in range(2):
            nc.gpsimd.dma_start(wq[dc][:], prm[f"wqT{l}"][dc * P:(dc + 1) * P, :])
        w["q"] = wq
        for kind in ("k", "v"):
            tiles = []
            for r in range(R):
                rt = [sb.tile([P, 256], BF, tag=f"w{kind}", bufs=2 * R, name=f"w{kind}{l}_{r}_{dc}") for dc in range(2)]
                for dc in range(2):
                    nc.gpsimd.dma_start(rt[dc][:], prm[f"w{kind}T{l}"][r, dc * P:(dc + 1) * P, :])
                tiles.append(rt)
            w[kind] = tiles
        f1 = [sb.tile([P, DFF], BF, tag="f1", bufs=2, name=f"f1{l}_{dc}") for dc in range(2)]
        for dc in range(2):
            nc.gpsimd.dma_start(f1[dc][:], prm[f"f1{l}"][dc * P:(dc + 1) * P, :])
        w["f1"] = f1
        f2 = [sb.tile([P, 256], BF, tag="f2", bufs=8, name=f"f2{l}_{dc}") for dc in range(8)]
        for dc in range(8):
            nc.gpsimd.dma_start(f2[dc][:], prm[f"f2{l}"][dc * P:(dc + 1) * P, :])
        w["f2"] = f2
        for nm, pn in (("f1b", f"f1b{l}"), ("f2b", f"f2b{l}"), ("g1", f"ln1_g{l}"), ("b1", f"ln1_b{l}"),
                       ("g2", f"ln2_g{l}"), ("b2", f"ln2_b{l}")):
            dim = DFF if nm == "f1b" else D
            nchunks = dim // P
            t = [sb.tile([P, 1], F32, tag="bias", bufs=44, name=f"{nm}{l}_{c}") for c in range(nchunks)]
            for c in range(nchunks):
                nc.sync.dma_start(t[c][:], prm[pn][c * P:(c + 1) * P, None])
            w[nm] = t
        return w

    def layernorm(T, g, b, lid):
        """Per-chunk LN over the feature axis (256, on partitions).
        T: [2] tiles [128, M] bf16 -> returns new [2] bf16 tiles (tag xT)."""
        out = [sb.tile([P, M], BF, tag="xT", bufs=3, name=f"lnout{lid}_{c}") for c in range(2)]
        for k in range(NCH):
            cs = slice(k * CB, (k + 1) * CB)
            mu_ps = ps.tile([1, CB], F32, tag="pA", bufs=2, name=f"mups{lid}_{k}")
            sq_ps = ps.tile([1, CB], F32, tag="pB", bufs=2, name=f"sqps{lid}_{k}")
            for c in range(2):
                tsq = sb.tile([P, CB], BF, tag="tsq", bufs=2, name=f"tsq{lid}_{k}_{c}")
                nc.scalar.activation(out=tsq[:], in_=T[c][:, cs], func=AF.Square)
                nc.tensor.matmul(out=mu_ps[:], lhsT=inv256[:], rhs=T[c][:, cs], start=(c == 0), stop=(c == 1))
                nc.tensor.matmul(out=sq_ps[:], lhsT=inv256[:], rhs=tsq[:], start=(c == 0), stop=(c == 1))
            mu_sb = sb.tile([1, CB], F32, tag="lnf32", bufs=8, name=f"musb{lid}_{k}")
            nc.scalar.copy(mu_sb[:], mu_ps[:])
            musq = sb.tile([1, CB], F32, tag="lnf32", bufs=8, name=f"musq{lid}_{k}")
            nc.vector.scalar_tensor_tensor(out=musq[:], in0=mu_sb[:], scalar=1.0, in1=mu_sb[:],
                                           op0=OP.bypass, op1=OP.mult)
            var = sb.tile([1, CB], F32, tag="lnf32", bufs=8, name=f"var{lid}_{k}")
            nc.vector.scalar_tensor_tensor(out=var[:], in0=sq_ps[:], scalar=1.0, in1=musq[:],
                                           op0=OP.bypass, op1=OP.subtract)
            sd = sb.tile([1, CB], F32, tag="lnf32", bufs=8, name=f"sd{lid}_{k}")
            nc.scalar.activation(out=sd[:], in_=var[:], func=AF.Sqrt, bias=eps_t[:])
            rstd32 = sb.tile([1, CB], F32, tag="lnf32", bufs=8, name=f"rstd32{lid}_{k}")
            nc.vector.reciprocal(rstd32[:], sd[:])
            rstd_bf = sb.tile([1, CB], BF, tag="lnbf", bufs=4, name=f"rstdbf{lid}_{k}")
            nc.vector.tensor_copy(rstd_bf[:], rstd32[:])
            mu_bf = sb.tile([1, CB], BF, tag="lnbf", bufs=4, name=f"mubf{lid}_{k}")
            nc.vector.tensor_copy(mu_bf[:], mu_sb[:])
            mu_bc = ps.tile([P, CB], F32, tag="pC", bufs=2, name=f"mubc{lid}_{k}")
            rs_bc = ps.tile([P, CB], F32, tag="pD", bufs=2, name=f"rsbc{lid}_{k}")
            nc.tensor.matmul(out=mu_bc[:], lhsT=ones1m[:], rhs=mu_bf[:], start=True, stop=True)
            nc.tensor.matmul(out=rs_bc[:], lhsT=ones1m[:], rhs=rstd_bf[:], start=True, stop=True)
            for c in range(2):
                u = sb.tile([P, CB], BF, tag="lnu", bufs=2, name=f"u{lid}_{k}_{c}")
                nc.vector.tensor_tensor(out=u[:], in0=T[c][:, cs], in1=mu_bc[:], op=OP.subtract)
                w_ = sb.tile([P, CB], BF, tag="lnu", bufs=2, name=f"w{lid}_{k}_{c}")
                nc.vector.tensor_tensor(out=w_[:], in0=u[:], in1=rs_bc[:], op=OP.mult)
                nc.scalar.activation(out=out[c][:, cs], in_=w_[:], func=AF.Identity, bias=b[c][:], scale=g[c][:])
        return out

    for l in range(L):
        w = load_w(l)
        if l == 0:
            xfull = prm["xfull0"]
        else:
            # AG-prep: PE-transpose xT -> row tiles, DMA to bounce, AllGather
            ag_in = dr.tile([M, D], BF, name=f"agin{l}")
            for t in range(64):
                rt_ps = ps.tile([P, D], BF, tag="pC", bufs=2, name=f"rtps{l}_{t}")
                for c in range(2):
                    nc.tensor.transpose(out=rt_ps[:, c * P:(c + 1) * P],
                                        in_=xT[c][:, t * P:(t + 1) * P], identity=ident[:])
                rows = sb.tile([P, D], BF, tag="rows", bufs=3, name=f"rows{l}_{t}")
                nc.scalar.copy(rows[:], rt_ps[:])
                nc.sync.dma_start(ag_in[t * P:(t + 1) * P, :], rows[:])
            ag_out = dr.tile([N, D], BF, addr_space="Shared", name=f"agout{l}")
            nc.gpsimd.collective_compute(
                "AllGather", OP.bypass,
                replica_groups=[list(range(NCORES))],
                ins=[ag_in.opt()], outs=[ag_out.opt()],
            )
            xfull = ag_out
            if l == 1 and dbg:
                gi = sb.tile([P, D], BF, tag="dbgg", bufs=1, name="dbg_gi")
                nc.sync.dma_start(gi[:], ag_in[0:P, :])
                dump("agin0", gi[:])
                go = sb.tile([P, D], BF, tag="dbgg", bufs=1, name="dbg_go")
                nc.sync.dma_start(go[:], ag_out[0:P, :])
                dump("agout0", go[:])
                gc4 = sb.tile([P, D], BF, tag="dbgg", bufs=1, name="dbg_gc4")
                nc.sync.dma_start(gc4[:], ag_out[4 * M:4 * M + P, :])
                dump("agoutc4", gc4[:])

        xnscr = dr.tile([8, P, M], BF, name=f"xnscr{l}")
        E_dram = dr.tile([R, 2, 2, M], BF, name=f"edram{l}")  # [r, oc, 2, M]

        # ---------------- K pass: gather + Q/K/logits/E
        Sparts = {}
        for r in range(R):
            for oc in range(2):
                Sparts[(r, oc)] = sb.tile([2, NCH], F32, tag="Sp", bufs=10, name=f"Sp{l}_{r}_{oc}")
        for k in range(NCH):
            cs = slice(k * CB, (k + 1) * CB)
            xnb = {}
            for r in range(4):
                for c in range(2):
                    xnb[(r, c)] = sb.tile([P, CB], BF, tag="xnb", bufs=10, name=f"xnb{l}_{k}_{r}_{c}")
                for s in range(4):
                    g = sb.tile([P, D], BF, tag="g", bufs=6, name=f"g{l}_{k}_{r}_{s}")
                    j = k * 16 + r * 4 + s
                    nc.gpsimd.indirect_dma_start(
                        out=g[:], out_offset=None, in_=xfull[:],
                        in_offset=bass.IndirectOffsetOnAxis(ap=idx_t[:, j:j + 1], axis=0))
                    for c in range(2):
                        nc.sync.dma_start_transpose(xnb[(r, c)][:, s * P:(s + 1) * P], g[:, c * P:(c + 1) * P])
                for c in range(2):
                    nc.sync.dma_start(xnscr[r * 2 + c, :, cs], xnb[(r, c)][:])
                if l == 0 and k == 0 and r == 0:
                    dump("xnb00", xnb[(0, 0)][:])
                if l == 1 and k == 0 and r == 0:
                    dump("xnb10", xnb[(0, 0)][:])
            q_sb = {}
            for oc in range(2):
                q_ps = ps.tile([P, CB], F32, tag="pA", bufs=2, name=f"qps{l}_{k}_{oc}")
                for dc in range(2):
                    nc.tensor.matmul(out=q_ps[:], lhsT=w["q"][dc][:, oc * P:(oc + 1) * P],
                                     rhs=xT[dc][:, cs], start=(dc == 0), stop=(dc == 1))
                q_sb[oc] = sb.tile([P, CB], BF, tag="qsb", bufs=3, name=f"qsb{l}_{k}_{oc}")
                nc.scalar.copy(q_sb[oc][:], q_ps[:])
                if l == 0 and k == 0 and oc == 0:
                    dump("qsb0", q_sb[0][:])
            for r in range(R):
                for oc in range(2):
                    k_ps = ps.tile([P, CB], F32, tag="pC", bufs=2, name=f"kps{l}_{k}_{r}_{oc}")
                    for dc in range(2):
                        rhs = xT[dc][:, cs] if r == 0 else xnb[(r - 1, dc)][:]
                        nc.tensor.matmul(out=k_ps[:], lhsT=w["k"][r][dc][:, oc * P:(oc + 1) * P],
                                         rhs=rhs, start=(dc == 0), stop=(dc == 1))
                    prod = sb.tile([P, CB], BF, tag="prod", bufs=2, name=f"prod{l}_{k}_{r}_{oc}")
                    nc.vector.tensor_tensor(out=prod[:], in0=q_sb[oc][:], in1=k_ps[:], op=OP.mult)
                    lg_ps = ps.tile([2, CB], F32, tag="pD", bufs=2, name=f"lg{l}_{k}_{r}_{oc}")
                    nc.tensor.matmul(out=lg_ps[:], lhsT=ones2h[:], rhs=prod[:], start=True, stop=True)
                    e_c = sb.tile([2, CB], BF, tag="ec", bufs=4, name=f"ec{l}_{k}_{r}_{oc}")
                    nc.scalar.activation(out=e_c[:], in_=lg_ps[:], func=AF.Exp,
                                         scale=SCALE, accum_out=Sparts[(r, oc)][:, k:k + 1])
                    nc.sync.dma_start(E_dram[r, oc, :, cs], e_c[:])
                    if l == 0 and k == 0 and r == 0 and oc == 0:
                        dump("prod0", prod[:])
                        dump("ec0", e_c[:])

        # ---------------- softmax-sum AllReduce
        s_in = dr.tile([20], F32, name=f"sin{l}")
        s_out = dr.tile([20], F32, addr_space="Shared", name=f"sout{l}")
        for r in range(R):
            for oc in range(2):
                sl = sb.tile([2, 1], F32, tag="sloc", bufs=10, name=f"sloc{l}_{r}_{oc}")
                nc.vector.tensor_reduce(out=sl[:], in_=Sparts[(r, oc)][:], axis=mybir.AxisListType.X, op=OP.add)
                nc.sync.dma_start(s_in[(r * 4 + oc * 2):(r * 4 + oc * 2 + 2), None], sl[:])
        nc.gpsimd.collective_compute(
            "AllReduce", OP.add,
            replica_groups=[list(range(NCORES))],
            ins=[s_in.opt()], outs=[s_out.opt()],
        )
        albc = {}
        for r in range(R):
            for oc in range(2):
                sg = sb.tile([2, 1], F32, tag="sg", bufs=10, name=f"sg{l}_{r}_{oc}")
                nc.sync.dma_start(sg[:], s_out[(r * 4 + oc * 2):(r * 4 + oc * 2 + 2), None])
                rs = sb.tile([2, 1], F32, tag="rs", bufs=10, name=f"rs{l}_{r}_{oc}")
                nc.vector.reciprocal(rs[:], sg[:])
                ab = sb.tile([2, P], BF, tag="ab", bufs=10, name=f"ab{l}_{r}_{oc}")
                nc.vector.tensor_scalar(out=ab[:], in0=ind2[:], scalar1=rs[:], scalar2=None, op0=OP.mult)
                albc[(r, oc)] = ab
                if l == 0 and r == 0 and oc == 0:
                    dump("sp00", Sparts[(0, 0)][:])
                    dump("ab0", ab[:])

        # ---------------- V pass + attention-out accumulation into T1 = x + Z
        T1 = [sb.tile([P, M], BF, tag="T", bufs=2, name=f"T1{l}_{c}") for c in range(2)]
        for k in range(NCH):
            cs = slice(k * CB, (k + 1) * CB)
            xnb2 = {}
            for rc in range(8):
                t = sb.tile([P, CB], BF, tag="xnb2", bufs=10, name=f"xnb2{l}_{k}_{rc}")
                nc.sync.dma_start(t[:], xnscr[rc, :, cs])
                xnb2[rc] = t
            for oc in range(2):
                acc_prev = None
                for r in range(R):
                    v_ps = ps.tile([P, CB], F32, tag="pA", bufs=2, name=f"vps{l}_{k}_{r}_{oc}")
                    for dc in range(2):
                        rhs = xT[dc][:, cs] if r == 0 else xnb2[(r - 1) * 2 + dc][:]
                        nc.tensor.matmul(out=v_ps[:], lhsT=w["v"][r][dc][:, oc * P:(oc + 1) * P],
                                         rhs=rhs, start=(dc == 0), stop=(dc == 1))
                    e2 = sb.tile([2, CB], BF, tag="e2", bufs=4, name=f"e2{l}_{k}_{r}_{oc}")
                    nc.sync.dma_start(e2[:], E_dram[r, oc, :, cs])
                    abc_ps = ps.tile([P, CB], F32, tag="pB", bufs=2, name=f"abc{l}_{k}_{r}_{oc}")
                    nc.tensor.matmul(out=abc_ps[:], lhsT=albc[(r, oc)][:], rhs=e2[:], start=True, stop=True)
                    abc_sb = sb.tile([P, CB], BF, tag="absb", bufs=3, name=f"absb{l}_{k}_{r}_{oc}")
                    nc.scalar.copy(abc_sb[:], abc_ps[:])
                    tmp = sb.tile([P, CB], BF, tag="ztmp", bufs=3, name=f"zt{l}_{k}_{r}_{oc}")
                    nc.vector.tensor_tensor(out=tmp[:], in0=v_ps[:], in1=abc_sb[:], op=OP.mult)
                    if l == 0 and k == 0 and r == 0 and oc == 0:
                        dump("absb0", abc_sb[:])
                        dump("tmp0", tmp[:])
                    if r == 0:
                        nxt = sb.tile([P, CB], BF, tag="zacc", bufs=4, name=f"za{l}_{k}_{r}_{oc}")
                        nc.vector.tensor_tensor(out=nxt[:], in0=xT[oc][:, cs], in1=tmp[:], op=OP.add)
                        acc_prev = nxt
                    elif r < R - 1:
                        nxt = sb.tile([P, CB], BF, tag="zacc", bufs=4, name=f"za{l}_{k}_{r}_{oc}")
                        nc.vector.tensor_tensor(out=nxt[:], in0=acc_prev[:], in1=tmp[:], op=OP.add)
                        acc_prev = nxt
                    else:
                        nc.vector.tensor_tensor(out=T1[oc][:, cs], in0=acc_prev[:], in1=tmp[:], op=OP.add)
                        if l == 0 and k == 0 and oc == 0:
                            dump("T1c0", T1[0][:, 0:CB])

        xn1 = layernorm(T1, w["g1"], w["b1"], lid=f"{l}a")
        if l == 0:
            dump("xn1c0", xn1[0][:, 0:CB])

        # ---------------- FFN + residual + LN2
        T2 = [sb.tile([P, M], BF, tag="T", bufs=2, name=f"T2{l}_{c}") for c in range(2)]
        for k in range(NCH):
            cs = slice(k * CB, (k + 1) * CB)
            h = []
            for oc in range(8):
                h_ps = ps.tile([P, CB], F32, tag="pA", bufs=2, name=f"f1ps{l}_{k}_{oc}")
                for dc in range(2):
                    nc.tensor.matmul(out=h_ps[:], lhsT=w["f1"][dc][:, oc * P:(oc + 1) * P],
                                     rhs=xn1[dc][:, cs], start=(dc == 0), stop=(dc == 1))
                ht = sb.tile([P, CB], BF, tag="h", bufs=9, name=f"h{l}_{k}_{oc}")
                nc.scalar.activation(out=ht[:], in_=h_ps[:], func=AF.Relu, bias=w["f1b"][oc][:])
                h.append(ht)
                if l == 0 and k == 0 and oc == 0:
                    dump("hc0", ht[:])
            for oc in range(2):
                h2_ps = ps.tile([P, CB], F32, tag="pB", bufs=2, name=f"f2ps{l}_{k}_{oc}")
                for dc in range(8):
                    nc.tensor.matmul(out=h2_ps[:], lhsT=w["f2"][dc][:, oc * P:(oc + 1) * P],
                                     rhs=h[dc][:], start=(dc == 0), stop=(dc == 7))
                nc.vector.scalar_tensor_tensor(out=T2[oc][:, cs], in0=h2_ps[:], scalar=w["f2b"][oc][:],
                                               in1=xn1[oc][:, cs], op0=OP.add, op1=OP.add)
        if l == 0:
            dump("T2c0", T2[0][:, 0:CB])
        xT = layernorm(T2, w["g2"], w["b2"], lid=f"{l}b")
        if l == 0:
            dump("xTn0", xT[0][:, 0:CB])

    # ---------------- heads
    pw1 = [sb.tile([P, PDIM], BF, tag="pw1", bufs=2, name=f"pw1_{dc}") for dc in range(2)]
    for dc in range(2):
        nc.gpsimd.dma_start(pw1[dc][:], prm["pw1T"][dc * P:(dc + 1) * P, :])
    pw2 = [sb.tile([P, MOVES], BF, tag="pw2", bufs=4, name=f"pw2_{dc}") for dc in range(4)]
    for dc in range(4):
        nc.gpsimd.dma_start(pw2[dc][:], prm["pw2T"][dc * P:(dc + 1) * P, :])
    pb1 = [sb.tile([P, 1], F32, tag="bias", bufs=44, name=f"pb1_{c}") for c in range(4)]
    for c in range(4):
        nc.sync.dma_start(pb1[c][:], prm["pb1"][c * P:(c + 1) * P, None])
    pb2t = sb.tile([MOVES, 1], F32, tag="pb2", bufs=1, name="pb2t")
    nc.sync.dma_start(pb2t[:], prm["pb2"][:, None])
    for k in range(NCH):
        cs = slice(k * CB, (k + 1) * CB)
        hp = []
        for oc in range(4):
            hp_ps = ps.tile([P, CB], F32, tag="pA", bufs=2, name=f"pps{k}_{oc}")
            for dc in range(2):
                nc.tensor.matmul(out=hp_ps[:], lhsT=pw1[dc][:, oc * P:(oc + 1) * P],
                                 rhs=xT[dc][:, cs], start=(dc == 0), stop=(dc == 1))
            t = sb.tile([P, CB], BF, tag="hp", bufs=5, name=f"hp{k}_{oc}")
            nc.scalar.activation(out=t[:], in_=hp_ps[:], func=AF.Relu, bias=pb1[oc][:])
            hp.append(t)
        p2_ps = ps.tile([MOVES, CB], F32, tag="pB", bufs=2, name=f"p2ps{k}")
        for dc in range(4):
            nc.tensor.matmul(out=p2_ps[:], lhsT=pw2[dc][:], rhs=hp[dc][:], start=(dc == 0), stop=(dc == 3))
        pol_c = sb.tile([MOVES, CB], F32, tag="polc", bufs=2, name=f"polc{k}")
        nc.vector.tensor_scalar(out=pol_c[:], in0=p2_ps[:], scalar1=pb2t[:], scalar2=None, op0=OP.add)
        nc.sync.dma_start(polT[:, cs], pol_c[:])

    # value head
    NG = B // NCORES
    vw1 = [sb.tile([P, VDIM], BF, tag="vw1", bufs=2, name=f"vw1_{dc}") for dc in range(2)]
    for dc in range(2):
        nc.gpsimd.dma_start(vw1[dc][:], prm["vw1T"][dc * P:(dc + 1) * P, :])
    vw2 = [sb.tile([P, 1], BF, tag="vw2", bufs=4, name=f"vw2_{dc}") for dc in range(4)]
    for dc in range(4):
        nc.gpsimd.dma_start(vw2[dc][:], prm["vw2T"][dc * P:(dc + 1) * P, :])
    vb1 = [sb.tile([P, 1], F32, tag="bias", bufs=44, name=f"vb1_{c}") for c in range(4)]
    for c in range(4):
        nc.sync.dma_start(vb1[c][:], prm["vb1"][c * P:(c + 1) * P, None])
    vb2t = sb.tile([1, 1], F32, tag="vb2", bufs=1, name="vb2t")
    nc.sync.dma_start(vb2t[:], prm["vb2"][:, None])
    pooled = [sb.tile([P, NG], BF, tag="pooled", bufs=2, name=f"pooled_{c}") for c in range(2)]
    for c in range(2):
        pf = sb.tile([P, NG], F32, tag="poolf", bufs=2, name=f"poolf_{c}")
        nc.vector.tensor_reduce(out=pf[:], in_=xT[c][:].rearrange("p (g n) -> p g n", n=NPG),
                                axis=mybir.AxisListType.X, op=OP.add)
        nc.vector.tensor_scalar(out=pooled[c][:], in0=pf[:], scalar1=1.0 / NPG, scalar2=None, op0=OP.mult)
    hv = []
    for oc in range(4):
        hv_ps = ps.tile([P, NG], F32, tag="pA", bufs=2, name=f"vps1_{oc}")
        for dc in range(2):
            nc.tensor.matmul(out=hv_ps[:], lhsT=vw1[dc][:, oc * P:(oc + 1) * P],
                             rhs=pooled[dc][:], start=(dc == 0), stop=(dc == 1))
        t = sb.tile([P, NG], BF, tag="hv", bufs=4, name=f"hv_{oc}")
        nc.scalar.activation(out=t[:], in_=hv_ps[:], func=AF.Relu, bias=vb1[oc][:])
        hv.append(t)
    v2_ps = ps.tile([1, NG], F32, tag="pB", bufs=2, name="vps2")
    for dc in range(4):
        nc.tensor.matmul(out=v2_ps[:], lhsT=vw2[dc][:], rhs=hv[dc][:], start=(dc == 0), stop=(dc == 3))
    vout = sb.tile([1, NG], F32, tag="vout", bufs=1, name="vout")
    nc.scalar.activation(out=vout[:], in_=v2_ps[:], func=AF.Tanh, bias=vb2t[:])
    nc.sync.dma_start(valT[:], vout[:])
    stack.close()


# ---------------------------------------------------------------- runner
def _get_nc():
    if "nc" not in _cached:
        _cached["nc"] = build_bass()
    return _cached["nc"]


def make_in_maps(inputs):
    inp = {k: np.asarray(v) for k, v in inputs.items()}
    dowker = inp["dowker"].astype(np.int64)
    ptr = inp["ptr"].astype(np.int64)
    adjacency = inp["adjacency"].astype(np.int64)
    assert np.all(ptr == np.arange(B + 1) * NPG), "kernel assumes uniform graphs"
    x0 = _host_x0(dowker, ptr)                      # [N, 256] f32
    x0_bf = x0.astype(ml_dtypes.bfloat16)
    w = _prep_weights(inp)

    bf16c = lambda a: np.ascontiguousarray(a).astype(ml_dtypes.bfloat16)
    c_ones1m = bf16c(np.ones((1, P), np.float32))
    c_inv256 = bf16c(np.full((P, 1), 1.0 / 256.0, np.float32))
    c_ones2h = np.zeros((P, 2), np.float32)
    c_ones2h[:64, 0] = 1.0
    c_ones2h[64:, 1] = 1.0
    c_ones2h = bf16c(c_ones2h)
    c_ind2 = np.zeros((2, P), np.float32)
    c_ind2[0, :64] = 1.0
    c_ind2[1, 64:] = 1.0
    c_ind2 = bf16c(c_ind2)

    in_maps = []
    for core in range(NCORES):
        shard = slice(core * M, (core + 1) * M)
        xs = x0_bf[shard]                            # [8192, 256]
        xT0 = np.stack([np.ascontiguousarray(xs[:, :P].T), np.ascontiguousarray(xs[:, P:].T)])
        adj = adjacency[shard].astype(np.int32)      # [8192, 4] global ids
        gidx = np.zeros((P, 256), np.int32)
        for nch in range(NCH):
            for r in range(4):
                for s in range(4):
                    j = nch * 16 + r * 4 + s
                    gidx[:, j] = adj[nch * 512 + s * 128: nch * 512 + s * 128 + 128, r]
        m = {"xT0": xT0, "xfull0": x0_bf, "gidx": gidx,
             "c_ones1m": c_ones1m, "c_inv256": c_inv256, "c_ones2h": c_ones2h, "c_ind2": c_ind2}
        for kk, v in w.items():
            m[kk] = v.astype(np.float32)
        in_maps.append(m)
    return in_maps


def assemble(results):
    policy = np.concatenate([np.ascontiguousarray(r["policyT"].T) for r in results], axis=0)
    value = np.concatenate([r["valT"][0] for r in results], axis=0)
    return policy.astype(np.float32), value.astype(np.float32)


def kernel(**inputs):
    nc = _get_nc()
    in_maps = make_in_maps(inputs)
    res = run_bass_kernel_spmd(nc, in_maps, core_ids=list(range(NCORES)))
    return assemble(res.results)
